# revision 2
# baseline (speedup 1.0000x reference)
# Trainium2 Bass kernel for nn_MinLoss_15229954032079.
#
# Math: loss = sum_b sum_s dist(p[b,s], g[b,match(b,s)]) / B, where
# dist is the euclidean distance between flattened [T*D] source signals
# and match is a greedy bipartite assignment on the [S,S] distance matrix.
#
# All pairwise distances derive from the 8x8 Gram matrix of the 8 flattened
# source vectors (4 prediction sources + 4 ground-truth sources) per batch:
#   d2[s,t] = G[s,s] + G[4+t,4+t] - 2*G[s,4+t]
#
# Strategy (one NeuronCore per batch element, 8 cores):
#   - The per-core stream (33.7 MB f32) is bound by per-SDMA-engine read
#     bandwidth (~26 GB/s x 16 engines ~= 425 GB/s -> ~79 us floor).
#   - Stream p[b], g[b] into SBUF as f32 via HWDGE (nc.sync) DMAs, one
#     contiguous DMA per (window, tensor). HWDGE avoids the SWDGE
#     descriptor-ring contention that made SDMA engine 15 ~11% slower
#     than the rest (a ~9 us straggler tail on the SWDGE cast path).
#   - The f32->bf16 cast happens in the shuffle copies instead, split
#     across two engines: DVE handles the p-half, ACT (scalar) the
#     g-half. Each window is shuffled into a blocked bf16 layout where
#     block r=(ti,dg) holds one column group of 16 consecutive d's per
#     source j, so every matmul operand is a contiguous 128-column slice.
#   - For each 128-column block, accumulate PSUM += block^T @ block on
#     the PE. PSUM entry (16j+u, 16j'+u) holds partial dot products of
#     sources j,j'; summing the 16 u-diagonals on the host yields the
#     exact 8x8 Gram. The d=256 leftover columns accumulate into a
#     second [32,32] PSUM (fixed width; unused columns of short windows
#     are zeroed so the accumulation stays clean).
#   - Windows taper (512x7, 256, 128, 128) so the serial tail after the
#     last DMA byte is one 128-row window's copies + 16 matmuls.
#   - Tiny [4,4] greedy matching + final scalar reduction on host.
#   - TileContext's exit sequence is patched to skip the per-semaphore
#     clear pass (each run executes a freshly loaded NEFF).

import numpy as np

B, T, S, D = 8, 4096, 4, 257
NCORES = 8
WSIZES = [512] * 7 + [256, 128, 128]  # time steps per window; sum == T
NW = len(WSIZES)
NMM = 16 * (sum(WSIZES) // 128)  # total body matmuls
PSB = 32  # tail psum operand width: (t<=4, half, j) -> t*8 + h*4 + j

_cached_nc = None


def _light_drain_and_barrier(self, tick_clock, wait_clock):
    # Replaces TileContext._drain_and_barrier: keep the drain + one
    # all-engine barrier, but skip the per-semaphore clear pass and the
    # second barrier (~6 us). Safe here because every kernel() invocation
    # executes a freshly loaded NEFF, so semaphores start from zero and
    # don't need to be restored for a re-run.
    from concourse.vector_clock import ScopedClock

    drain_inst = self.nc.sync.drain()
    wait_clock.add_sem_waits(
        drain_inst.ins, ScopedClock({None: tick_clock.global_clock})
    )
    self.nc.all_engine_barrier()
    popped = self.nc._tile_sem_poison_stack.pop()
    assert popped is self._sem_poison

def _build_nc():
    import concourse.bacc as bacc
    import concourse.tile as tile
    from concourse import mybir

    nc = bacc.Bacc("TRN2", target_bir_lowering=False, debug=False)
    p_dram = nc.dram_tensor("p", [T, S, D], mybir.dt.float32, kind="ExternalInput")
    g_dram = nc.dram_tensor("g", [T, S, D], mybir.dt.float32, kind="ExternalInput")
    gram_dram = nc.dram_tensor(
        "gram", [128, 128], mybir.dt.float32, kind="ExternalOutput"
    )
    gram2_dram = nc.dram_tensor(
        "gram2", [PSB, PSB], mybir.dt.float32, kind="ExternalOutput"
    )

    orig_drain = tile.TileContext._drain_and_barrier
    tile.TileContext._drain_and_barrier = _light_drain_and_barrier

    with tile.TileContext(nc) as tc:
        with (
            tc.tile_pool(name="slab", bufs=4) as fpool,
            tc.tile_pool(name="blk16", bufs=3) as bpool,
            tc.tile_pool(name="psum", bufs=1, space="PSUM") as ppool,
            tc.tile_pool(name="out", bufs=1) as opool,
        ):
            psa = ppool.tile([128, 128], mybir.dt.float32)
            psb = ppool.tile([PSB, PSB], mybir.dt.float32)

            mm_i = 0
            t0 = 0
            for w, ws in enumerate(WSIZES):
                ti = ws // 128
                half = ti * S * D  # f32 cols per tensor half in raw HBM order
                nblk = 16 * ti

                # Window rows [t0, t0+ws): partition p holds times
                # t0 + p*ti .. t0 + p*ti + ti - 1, fully contiguous per
                # partition (ti*4*257*4 bytes). HWDGE f32 landing.
                pv = p_dram.ap()[t0 : t0 + ws].rearrange(
                    "(p ti) s d -> p ti s d", p=128
                )
                gv = g_dram.ap()[t0 : t0 + ws].rearrange(
                    "(p ti) s d -> p ti s d", p=128
                )
                fsl = fpool.tile([128, 2 * half], mybir.dt.float32)
                nc.sync.dma_start(out=fsl[:, 0:half], in_=pv)
                nc.sync.dma_start(out=fsl[:, half : 2 * half], in_=gv)

                wb = bpool.tile([128, 128 * nblk + PSB], mybir.dt.bfloat16)
                # body blocked col: (ti*16+dg)*128 + j*16 + dl
                wv = wb[:, 0 : 128 * nblk].rearrange(
                    "p (ti dg j dl) -> p j ti dg dl", ti=ti, dg=16, j=8, dl=16
                )
                # tail region: fixed PSB=32 cols, col = t*8 + h*4 + j
                tb = wb[:, 128 * nblk : 128 * nblk + PSB].rearrange(
                    "p (t h j) -> p t h j", t=4, h=2, j=4
                )
                if ti < 4:
                    # zero the unused tail columns so the fixed-width psb
                    # accumulation sees no stale data
                    nc.vector.memset(
                        wb[:, 128 * nblk + 8 * ti : 128 * nblk + PSB], 0.0
                    )

                for h, base in ((0, 0), (1, half)):
                    hview = fsl[:, base : base + half].rearrange(
                        "p (ti c) -> p ti c", ti=ti
                    )
                    for jj in range(4):
                        src = hview[:, :, jj * D : jj * D + 256].rearrange(
                            "p ti (dg dl) -> p ti dg dl", dl=16
                        )
                        dst = wv[:, h * 4 + jj]
                        if h == 0:
                            nc.vector.tensor_copy(dst, src)
                        else:
                            nc.scalar.copy(dst, src)
                    # d=256 leftovers, one merged strided copy per half
                    tsrc = hview.rearrange("p ti (j d) -> p ti j d", j=4)[
                        :, :, :, 256
                    ]
                    if h == 0:
                        nc.vector.tensor_copy(tb[:, 0:ti, 0], tsrc)
                    else:
                        nc.scalar.copy(tb[:, 0:ti, 1], tsrc)

                for r in range(nblk):
                    blk = wb[:, 128 * r : 128 * (r + 1)]
                    nc.tensor.matmul(
                        psa[:], blk, blk, start=(mm_i == 0), stop=(mm_i == NMM - 1)
                    )
                    mm_i += 1
                tblk = wb[:, 128 * nblk : 128 * nblk + PSB]
                nc.tensor.matmul(
                    psb[:], tblk, tblk, start=(w == 0), stop=(w == NW - 1)
                )
                t0 += ws

            outt = opool.tile([128, 128], mybir.dt.float32)
            outt2 = opool.tile([PSB, PSB], mybir.dt.float32)
            nc.vector.tensor_copy(outt[:], psa[:])
            nc.vector.tensor_copy(outt2[:], psb[:])
            nc.sync.dma_start(out=gram_dram.ap(), in_=outt[:])
            nc.sync.dma_start(out=gram2_dram.ap(), in_=outt2[:])
    tile.TileContext._drain_and_barrier = orig_drain
    nc.compile()
    return nc


def _greedy_match_np(d):
    # replicate reference._greedy_match: repeated global argmin with
    # row/col masking; np.argmin matches jnp.argmin tie-breaking (first).
    s = d.shape[0]
    dm = d.astype(np.float32).copy()
    matches = np.zeros(s, np.int32)
    for _ in range(s):
        m = int(np.argmin(dm.reshape(-1)))
        r, c = divmod(m, s)
        matches[r] = c
        dm[r, :] = np.inf
        dm[:, c] = np.inf
    return matches


def _loss_from_gram(psa_list):
    total = 0.0
    for psa, psb in psa_list:
        # G8[j,k] = sum_u psa[16j+u, 16k+u] + sum_t psb[t*8+j, t*8+k]
        g8 = np.einsum("juku->jk", psa.reshape(8, 16, 8, 16).astype(np.float64))
        g8 += np.einsum("tjtk->jk", psb.reshape(4, 8, 4, 8).astype(np.float64))
        pn = np.diag(g8)[:4]
        gn = np.diag(g8)[4:]
        cr = g8[:4, 4:]
        d2 = pn[:, None] + gn[None, :] - 2.0 * cr
        dists = np.sqrt(np.maximum(d2, 0.0)).astype(np.float32)
        matches = _greedy_match_np(dists)
        total += float(dists[np.arange(4), matches].astype(np.float64).sum())
    return np.float32(total / B)


def kernel(**inputs):
    global _cached_nc
    preds = np.ascontiguousarray(inputs["predictions"], dtype=np.float32)
    gts = np.ascontiguousarray(inputs["ground_truths"], dtype=np.float32)
    assert preds.shape == (B, T, S, D) and gts.shape == (B, T, S, D)

    if _cached_nc is None:
        _cached_nc = _build_nc()
    nc = _cached_nc

    from concourse.bass_utils import run_bass_kernel_spmd

    in_maps = [{"p": preds[b], "g": gts[b]} for b in range(B)]
    res = run_bass_kernel_spmd(nc, in_maps, list(range(NCORES)))
    psa_list = [(res.results[b]["gram"], res.results[b]["gram2"]) for b in range(B)]
    return _loss_from_gram(psa_list)
